# revision 21
# baseline (speedup 1.0000x reference)
"""Trainium2 Bass kernel for IntrinsicSignalSynthesizer.

Data-parallel over 8 NeuronCores: batch 16384 -> 8 x 2048 rows.
Feature-major dataflow: activations live as [feature_chunk(128), rows] tiles so
every matmul contracts over the partition dim with zero on-device transposes.

v2: per-row reductions (entropy sums, |a|^2, MSE sum, and the three MLP head
dot-products) no longer burn 8 PE passes each; the chunk dimension is collapsed
on the DVE (tree adds / fused scalar_tensor_tensor chains) and a single
ones-vector matmul does the final 128-partition reduction.  PE work per tile
drops from ~142.6k to ~121.6k cycles; DVE (idle before) absorbs the rest.
"""
import sys
sys.path.insert(0, '/opt/trn_rl_repo')

import numpy as np
import ml_dtypes

import concourse.bass as bass
import concourse.bass_isa as bass_isa
import concourse.mybir as mybir
import concourse.tile as tile
from concourse.bass_utils import run_bass_kernel_spmd

BF16 = mybir.dt.bfloat16
F32 = mybir.dt.float32
AF = mybir.ActivationFunctionType
ALU = mybir.AluOpType

B, D = 16384, 1024
MEM = 100
NCORES = 8
ROWS = B // NCORES            # 2048 rows per core
NT = 512                      # rows per row-tile
NTILES = ROWS // NT           # 4
KD = D // 128                 # 8 feature chunks of prediction/actual

MAX_WAITS = 1


def _split_excess_waits(nc):
    # walrus CTRL encoding caps sync waits per instruction; the TileContext
    # tail drain can exceed that. Move excess waits onto preceding NoOps.
    for fn in nc.m.functions:
        for bb in fn.blocks:
            if not isinstance(bb, mybir.BasicBlock):
                continue
            insts = bb.instructions
            i = 0
            while i < len(insts):
                ins = insts[i]
                si = getattr(ins, 'sync_info', None)
                waits = list(si.on_wait) if si is not None and si.on_wait else []
                if len(waits) > MAX_WAITS:
                    chunks = [waits[j:j + MAX_WAITS]
                              for j in range(0, len(waits), MAX_WAITS)]
                    si.on_wait = chunks[-1]
                    new_ops = [
                        mybir.InstNoOp(
                            name=f"{ins.name}-waitsplit-{k}",
                            engine=ins.engine,
                            sync_info=mybir.SyncInfo(on_wait=ch, on_update=[]),
                            bass_nofuse=True,
                        )
                        for k, ch in enumerate(chunks[:-1])
                    ]
                    insts[i:i] = new_ops
                    i += len(new_ops)
                i += 1


def _mlp_layer(nc, pools, w_sb, bias_sb, x_tiles, kchunks, ofchunks, out_sb):
    """h = relu(W @ x + b) in feature-major layout.

    w_sb: [128, kchunks, 128*ofchunks] bf16; x_tiles(k) -> [128, NT] rhs AP;
    out_sb: [128, ofchunks, NT] bf16.
    """
    for j in range(ofchunks):
        ps = pools['mm'].tile([128, NT], F32, tag="mm")
        for k in range(kchunks):
            nc.tensor.matmul(ps, w_sb[:, k, j * 128:(j + 1) * 128], x_tiles(k),
                             start=(k == 0), stop=(k == kchunks - 1))
        nc.scalar.activation(out_sb[:, j, :], ps, AF.Relu,
                             bias=bias_sb[:, j:j + 1])


def _tree8(nc, pools, x, name):
    """Collapse the 8-chunk axis of x [128, 8, NT] bf16 -> [128, NT] bf16
    with 7 DVE adds (3-level tree). Scratch levels share one buffer across
    all trees (DVE executes them serially anyway)."""
    l1 = pools['big'].tile([128, 4, NT], BF16, tag="tr_l1", name="tr_l1")
    for i in range(4):
        nc.vector.tensor_tensor(l1[:, i, :], x[:, 2 * i, :], x[:, 2 * i + 1, :],
                                ALU.add)
    l2 = pools['sm'].tile([128, 2, NT], BF16, tag="tr_l2", name="tr_l2")
    for i in range(2):
        nc.vector.tensor_tensor(l2[:, i, :], l1[:, 2 * i, :], l1[:, 2 * i + 1, :],
                                ALU.add)
    out = pools['sm'].tile([128, NT], BF16, tag=name + "_s", name=name + "_s")
    nc.vector.tensor_tensor(out, l2[:, 0, :], l2[:, 1, :], ALU.add)
    return out


def _ones_mm(nc, pools, ones, x):
    """[1, NT] = column-sum of x [128, NT] via single PE pass."""
    ps = pools['vec'].tile([1, NT], F32, tag="vec")
    nc.tensor.matmul(ps, ones, x, start=True, stop=True)
    return ps


def _head_collapse(nc, pools, wf, h_sb, chunks, name):
    """DVE scalar_tensor_tensor chain: weighted chunk-collapse of h [128, c, NT]
    with per-partition weights wf [128, c] -> [128, NT] bf16.  The final
    partition reduction is a separate single ones-matmul (PE tail)."""
    acc = [pools['sm'].tile([128, NT], F32, tag="h_acc0", name="h_acc0"),
           pools['sm'].tile([128, NT], F32, tag="h_acc1", name="h_acc1")]
    nc.vector.tensor_scalar_mul(acc[0], h_sb[:, 0, :], wf[:, 0:1])
    accb = pools['sm'].tile([128, NT], BF16, tag=name, name=name)
    for j in range(1, chunks):
        out = accb if j == chunks - 1 else acc[j % 2]
        nc.vector.scalar_tensor_tensor(out, h_sb[:, j, :], wf[:, j:j + 1],
                                       acc[(j + 1) % 2],
                                       op0=ALU.mult, op1=ALU.add)
    return accb


def _softplus(nc, pools, z_ps, bias_sb, out_tag):
    """softplus(z + b) = Ln(1 + Exp(z + b)); both funcs share one ACT table set."""
    e = pools['sm'].tile([1, NT], F32, tag=out_tag + "_e")
    nc.scalar.activation(e, z_ps, AF.Exp, bias=bias_sb[0:1, 0:1])
    sp = pools['sm'].tile([1, NT], F32, tag=out_tag)
    nc.scalar.activation(sp, e, AF.Ln, bias=1.0)
    return sp


def build_kernel(reps: int = 1, hw_loop: bool = False):
    nc = bass.Bass()

    pt_d = nc.dram_tensor("pt", [D, ROWS], BF16, kind="ExternalInput")
    at_d = nc.dram_tensor("at", [D, ROWS], BF16, kind="ExternalInput")
    wd_d = nc.dram_tensor("wd", [2 * D, D], BF16, kind="ExternalInput")
    wu_d = nc.dram_tensor("wu", [D, D // 2], BF16, kind="ExternalInput")
    wn_d = nc.dram_tensor("wn", [D, D // 2], BF16, kind="ExternalInput")
    wc1_d = nc.dram_tensor("wc1", [D, D // 4], BF16, kind="ExternalInput")
    wc2_d = nc.dram_tensor("wc2", [D // 4, D], BF16, kind="ExternalInput")
    wh_d = nc.dram_tensor("wh", [128, 16], F32, kind="ExternalInput")  # d2|u2|n2
    mh_d = nc.dram_tensor("mh", [D, MEM], BF16, kind="ExternalInput")
    ones_d = nc.dram_tensor("ones", [128, 1], BF16, kind="ExternalInput")
    ident_d = nc.dram_tensor("ident", [128, 128], F32, kind="ExternalInput")
    bd1_d = nc.dram_tensor("bd1", [128, KD], F32, kind="ExternalInput")
    bu1_d = nc.dram_tensor("bu1", [128, 4], F32, kind="ExternalInput")
    bn1_d = nc.dram_tensor("bn1", [128, 4], F32, kind="ExternalInput")
    bc1_d = nc.dram_tensor("bc1", [128, 2], F32, kind="ExternalInput")
    bc2_d = nc.dram_tensor("bc2", [128, KD], F32, kind="ExternalInput")
    bh_d = nc.dram_tensor("bh", [1, 3], F32, kind="ExternalInput")  # d2,u2,n2
    out_d = nc.dram_tensor("out", [4, ROWS], F32, kind="ExternalOutput")

    with tile.TileContext(nc) as tc:
        pools = {}
        import contextlib
        ctx = contextlib.ExitStack()
        with ctx:
            W = ctx.enter_context(tc.tile_pool(name="weights", bufs=1))
            pools['io'] = ctx.enter_context(tc.tile_pool(name="io", bufs=2))
            pools['big'] = ctx.enter_context(tc.tile_pool(name="big", bufs=1))
            pools['sm'] = ctx.enter_context(tc.tile_pool(name="sm", bufs=1))
            pools['mm'] = ctx.enter_context(
                tc.tile_pool(name="mmp", bufs=3, space="PSUM"))
            pools['vec'] = ctx.enter_context(
                tc.tile_pool(name="vecp", bufs=3, space="PSUM"))
            pools['simp'] = ctx.enter_context(
                tc.tile_pool(name="simp", bufs=1, space="PSUM"))
            pools['tr'] = ctx.enter_context(
                tc.tile_pool(name="trp", bufs=1, space="PSUM"))

            # resident weights (loaded once). wd arrives in two halves so
            # the first d1 matmuls (which read only the prediction-half
            # chunks) can start after 2MB instead of 4MB of weight DMA.
            wd = W.tile([128, 16, D], BF16)
            nc.sync.dma_start(wd[:, 0:8, :],
                              wd_d[0:D, :].rearrange("(k p) m -> p k m", p=128))
            bd1 = W.tile([128, KD], F32)
            nc.sync.dma_start(bd1, bd1_d[:])
            nc.sync.dma_start(wd[:, 8:16, :],
                              wd_d[D:2 * D, :].rearrange("(k p) m -> p k m",
                                                         p=128))
            wu = W.tile([128, KD, D // 2], BF16)
            nc.sync.dma_start(wu, wu_d.rearrange("(k p) m -> p k m", p=128))
            wn = W.tile([128, KD, D // 2], BF16)
            nc.sync.dma_start(wn, wn_d.rearrange("(k p) m -> p k m", p=128))
            wc1 = W.tile([128, KD, D // 4], BF16)
            nc.sync.dma_start(wc1, wc1_d.rearrange("(k p) m -> p k m", p=128))
            wc2 = W.tile([128, 2, D], BF16)
            nc.sync.dma_start(wc2, wc2_d.rearrange("(k p) m -> p k m", p=128))
            wh = W.tile([128, 16], F32)   # wd2f[0:8] | wu2f[8:12] | wn2f[12:16]
            nc.sync.dma_start(wh, wh_d[:])
            mh = W.tile([128, KD, MEM], BF16)
            nc.sync.dma_start(mh, mh_d.rearrange("(k p) m -> p k m", p=128))
            ones = W.tile([128, 1], BF16)
            nc.sync.dma_start(ones, ones_d[:])
            ident = W.tile([128, 128], F32)
            nc.sync.dma_start(ident, ident_d[:])
            bu1 = W.tile([128, 4], F32)
            nc.sync.dma_start(bu1, bu1_d[:])
            bn1 = W.tile([128, 4], F32)
            nc.sync.dma_start(bn1, bn1_d[:])
            bc1 = W.tile([128, 2], F32)
            nc.sync.dma_start(bc1, bc1_d[:])
            bc2 = W.tile([128, KD], F32)
            nc.sync.dma_start(bc2, bc2_d[:])
            bh = W.tile([1, 3], F32)
            nc.sync.dma_start(bh, bh_d[:])

            def body():
                for t in range(NTILES):
                    rs = slice(t * NT, (t + 1) * NT)

                    pt = pools['io'].tile([128, KD, NT], BF16, tag="pt")
                    nc.sync.dma_start(
                        pt, pt_d[:, rs].rearrange("(k p) r -> p k r", p=128))
                    at = pools['io'].tile([128, KD, NT], BF16, tag="at")
                    nc.sync.dma_start(
                        at, at_d[:, rs].rearrange("(k p) r -> p k r", p=128))

                    # Emission order keeps each engine queue fed in
                    # producer-ready order: PE runs the big matmul groups
                    # back-to-back; the seven 1-row reductions form the PE
                    # tail; DVE sees |a|^2 first (needs only the DMA), then
                    # entropy (needs ACT exp), then the c2-coupled dj ops,
                    # then the head collapses (inputs ready long before).

                    # --- PE phase 1: dissonance layer 1
                    hd = pools['big'].tile([128, KD, NT], BF16, tag="hd")
                    _mlp_layer(nc, pools, wd, bd1,
                               lambda k: pt[:, k, :] if k < KD else at[:, k - KD, :],
                               16, KD, hd)
                    e = pools['big'].tile([128, KD, NT], BF16, tag="e")
                    nc.scalar.activation(e, pt, AF.Exp)

                    asq = pools['big'].tile([128, KD, NT], BF16, tag="asq")
                    nc.vector.tensor_mul(asq, at, at)
                    asum = _tree8(nc, pools, asq, "as")
                    ex = pools['big'].tile([128, KD, NT], BF16, tag="ex")
                    nc.vector.tensor_mul(ex, e, pt)
                    esum = _tree8(nc, pools, e, "es")
                    exsum = _tree8(nc, pools, ex, "exs")

                    # --- PE phase 2: uncertainty / novelty / compression L1
                    hu = pools['big'].tile([128, 4, NT], BF16, tag="hu")
                    _mlp_layer(nc, pools, wu, bu1, lambda k: pt[:, k, :],
                               KD, 4, hu)
                    hn = pools['big'].tile([128, 4, NT], BF16, tag="hn")
                    _mlp_layer(nc, pools, wn, bn1, lambda k: at[:, k, :],
                               KD, 4, hn)
                    hc = pools['big'].tile([128, 2, NT], BF16, tag="hc")
                    _mlp_layer(nc, pools, wc1, bc1, lambda k: pt[:, k, :],
                               KD, 2, hc)

                    # --- PE phase 3: memory sims, mh stationary:
                    # out [100 mem, NT rows]; max over mem via GPSIMD
                    # partition-reduce (frees PE of 24 row-block matmuls
                    # and the transpose).
                    pss = pools['simp'].tile([100, NT], F32, tag="simp")
                    for k in range(KD):
                        nc.tensor.matmul(pss, mh[:, k, :], at[:, k, :],
                                         start=(k == 0), stop=(k == KD - 1))
                    # max over the 100 memory rows (partition dim): pad to 128
                    # with -inf, transpose 128x128 blocks back to row-major,
                    # then a free-dim reduce_max per 128-row block.
                    pssb = pools['sm'].tile([128, NT], F32, tag="pssb")
                    nc.vector.memset(pssb[96:128, :], -1e30)
                    nc.scalar.copy(pssb[0:100, :], pss)
                    raw4 = pools['sm'].tile([128, 4], F32, tag="raw4")
                    for s in range(4):
                        pst = pools['tr'].tile([128, 128], F32, tag="tr")
                        nc.tensor.transpose(
                            pst, pssb[:, s * 128:(s + 1) * 128], ident)
                        nc.vector.reduce_max(raw4[:, s:s + 1], pst,
                                             axis=mybir.AxisListType.X)
                    pst4 = pools['tr'].tile([4, 128], F32, tag="tr")
                    nc.tensor.transpose(pst4, raw4, ident)
                    st = pools['sm'].tile([4, 128], F32, tag="st")
                    nc.scalar.copy(st, pst4)
                    mem_raw = pools['sm'].tile([1, 4, 128], F32, tag="mem_raw")
                    nc.sync.dma_start(mem_raw, st)

                    # --- PE phase 4: compression reconstruction
                    dsq = pools['big'].tile([128, KD, NT], BF16, tag="dsq")
                    for j in range(KD):
                        psr = pools['mm'].tile([128, NT], F32, tag="mm")
                        for k in range(2):
                            nc.tensor.matmul(psr, wc2[:, k, j * 128:(j + 1) * 128],
                                             hc[:, k, :],
                                             start=(k == 0), stop=(k == 1))
                        dj = pools['sm'].tile([128, NT], BF16, tag="dj")
                        # dj = (recon + bc2) - p   (sign-flipped diff; squared next)
                        nc.vector.scalar_tensor_tensor(
                            dj, psr, bc2[:, j:j + 1], pt[:, j, :],
                            op0=ALU.add, op1=ALU.subtract)
                        nc.vector.tensor_mul(dsq[:, j, :], dj, dj)
                    dsum = _tree8(nc, pools, dsq, "ds")

                    # head chunk-collapses (inputs long ready; DVE has slack
                    # here while PE finishes phase 4)
                    accd = _head_collapse(nc, pools, wh[:, 0:KD], hd, KD, 'accd')
                    accu = _head_collapse(nc, pools, wh[:, 8:12], hu, 4, 'accu')
                    accn = _head_collapse(nc, pools, wh[:, 12:16], hn, 4, 'accn')

                    # --- PE tail: the seven 1-row reductions
                    zZ = _ones_mm(nc, pools, ones, esum)
                    lnZ = pools['sm'].tile([1, NT], F32, tag="lnZ")
                    nc.scalar.activation(lnZ, zZ, AF.Ln)
                    iZ = pools['sm'].tile([1, NT], F32, tag="iZ")
                    nc.vector.reciprocal(iZ, zZ)
                    zS = _ones_mm(nc, pools, ones, exsum)
                    sz = pools['sm'].tile([1, NT], F32, tag="sz")
                    nc.vector.tensor_mul(sz, zS, iZ)
                    na2 = _ones_mm(nc, pools, ones, asum)
                    lnA = pools['sm'].tile([1, NT], F32, tag="lnA")
                    nc.scalar.activation(lnA, na2, AF.Ln)
                    zd = _ones_mm(nc, pools, ones, accd)
                    spD = _softplus(nc, pools, zd, bh[0:1, 0:1], "spD")
                    zu = _ones_mm(nc, pools, ones, accu)
                    spU = _softplus(nc, pools, zu, bh[0:1, 1:2], "spU")
                    zn = _ones_mm(nc, pools, ones, accn)
                    spN = _softplus(nc, pools, zn, bh[0:1, 2:3], "spN")
                    msum = _ones_mm(nc, pools, ones, dsum)

                    # --- aux epilogue (overlaps next tile's PE phase)
                    hent = pools['sm'].tile([1, NT], F32, tag="hent")
                    nc.vector.tensor_tensor(hent, lnZ, sz, ALU.subtract)
                    unc = pools['sm'].tile([1, NT], F32, tag="unc")
                    nc.vector.scalar_tensor_tensor(
                        unc, hent, 0.1, spU, op0=ALU.mult, op1=ALU.add)
                    ia = pools['sm'].tile([1, NT], F32, tag="ia")
                    nc.scalar.activation(ia, lnA, AF.Exp, scale=-0.5)
                    mr = mem_raw.rearrange("o s c -> o (s c)")
                    cos = pools['sm'].tile([1, NT], F32, tag="cos")
                    nc.vector.tensor_mul(cos, mr, ia)
                    # nov = 0.7*(1-cos) + 0.3*spN = (cos*-0.7) + (0.3*spN + 0.7)
                    spN3 = pools['sm'].tile([1, NT], F32, tag="spN3")
                    nc.vector.tensor_scalar(spN3, spN, 0.3, 0.7,
                                            op0=ALU.mult, op1=ALU.add)
                    nov = pools['sm'].tile([1, NT], F32, tag="nov")
                    nc.vector.scalar_tensor_tensor(
                        nov, cos, -0.7, spN3, op0=ALU.mult, op1=ALU.add)
                    comp = pools['sm'].tile([1, NT], F32, tag="comp")
                    nc.vector.tensor_scalar_mul(comp, msum, 1.0 / D)

                    nc.sync.dma_start(out_d[0:1, rs], spD)
                    nc.sync.dma_start(out_d[1:2, rs], unc)
                    nc.sync.dma_start(out_d[2:3, rs], nov)
                    nc.sync.dma_start(out_d[3:4, rs], comp)

            if hw_loop:
                with tc.For_i(0, reps):
                    body()
            else:
                for _ in range(reps):
                    body()

    _split_excess_waits(nc)
    return nc


def _prep_inputs(prediction, actual, pattern_memory,
                 W_d1, b_d1, W_d2, b_d2, W_u1, b_u1, W_u2, b_u2,
                 W_n1, b_n1, W_n2, b_n2, W_c1, b_c1, W_c2, b_c2):
    bf = ml_dtypes.bfloat16

    def t_bf(a):  # transposed contiguous bf16
        return np.ascontiguousarray(np.asarray(a, np.float32).T).astype(bf)

    mnorm = np.maximum(np.linalg.norm(
        np.asarray(pattern_memory, np.float32), axis=1), 1e-8)
    mhat = np.asarray(pattern_memory, np.float32) / mnorm[:, None]

    def fold(v, chunks):
        return np.ascontiguousarray(
            np.asarray(v, np.float32).reshape(chunks, 128).T)

    wh = np.concatenate([fold(W_d2[0], KD), fold(W_u2[0], 4),
                         fold(W_n2[0], 4)], axis=1)

    shared = {
        "wd": t_bf(W_d1), "wu": t_bf(W_u1), "wn": t_bf(W_n1),
        "wc1": t_bf(W_c1), "wc2": t_bf(W_c2),
        "wh": wh,
        "mh": t_bf(mhat),
        "ones": np.ones((128, 1), bf),
        "ident": np.eye(128, dtype=np.float32),
        "bd1": fold(b_d1, KD), "bu1": fold(b_u1, 4),
        "bn1": fold(b_n1, 4), "bc1": fold(b_c1, 2),
        "bc2": fold(b_c2, KD),
        "bh": np.asarray([[float(b_d2[0]), float(b_u2[0]), float(b_n2[0])]],
                         np.float32),
    }
    p32 = np.asarray(prediction, np.float32)
    a32 = np.asarray(actual, np.float32)
    in_maps = []
    for c in range(NCORES):
        rows = slice(c * ROWS, (c + 1) * ROWS)
        m = dict(shared)
        m["pt"] = np.ascontiguousarray(p32[rows].T).astype(bf)
        m["at"] = np.ascontiguousarray(a32[rows].T).astype(bf)
        in_maps.append(m)
    return in_maps


_NC_CACHE = {}


def kernel(**inputs) -> np.ndarray:
    in_maps = _prep_inputs(**inputs)
    if 'nc' not in _NC_CACHE:
        _NC_CACHE['nc'] = build_kernel(reps=1)
    nc = _NC_CACHE['nc']
    res = run_bass_kernel_spmd(nc, in_maps, core_ids=list(range(NCORES)))
    out = np.empty((B, 4), np.float32)
    for c in range(NCORES):
        out[c * ROWS:(c + 1) * ROWS, :] = res.results[c]["out"].T
    return out
